# revision 9
# baseline (speedup 1.0000x reference)
"""Chamfer distance kernel for Trainium2 (8 NeuronCores) — pruned NN search.

Problem: src [4, 8192, 3], tar [4, 8192, 3] fp32 ->
    chamfer [4] = 0.5 * (mean_m ||src_m - NN(tar)|| + mean_n ||tar_n - NN(src)||)

Sharding: 8 cores = 4 batches x 2 directions; each core solves one 8192-query
nearest-neighbor problem against 8192 targets.

Algorithm (KD-tree-style pruning; the device computes every candidate
distance and all mins):
  host: kd-median-split queries into 64 tiles of 128; per query, an upper
  bound on its NN distance from a strided spatial sample of targets; the
  tile's candidate set = targets inside the union of per-query balls
  (rasterized on a 48^3 grid). Candidate sets provably contain the true NN.
  device: per (query-tile, candidate-chunk) entry, a K=32 bf16 limb matmul
  produces s = |t|^2 - 2 q.t in PSUM; VectorE tensor_tensor_reduce(min)
  folds PSUM chunks against ScalarE-staged chunks into per-entry mins.
  host: d2 = entry-min grouped per tile + |q|^2 (fp64), sqrt, mean.

Precision: 3-limb bf16 splits of coordinates with all 9 limb products kept
(27 rows) + 5-limb bf16 split of |t|^2 (5 rows) = K 32. |q|^2 is added on
the host in fp64, which removes the argmin-bias of the baseline
(~2.7e-4 rel err in simulation vs 1.6e-2 for the old kernel).
"""

import sys
import numpy as np
import ml_dtypes


def _ensure_concourse():
    try:
        import concourse.bass  # noqa: F401
    except ImportError:
        for p in ("/opt/trn_rl_repo", "/root/.axon_site/_ro/trn_rl_repo"):
            if p not in sys.path:
                sys.path.insert(0, p)
        import concourse.bass  # noqa: F401


B = 4
N = 8192
K = 32
QTILE = 128
NQT = N // QTILE   # 64 query tiles
CHUNK = 512        # candidate columns per matmul / PSUM bank
MAXM = 4           # chunks per entry (PSUM tile = 4 banks)
NG = 48            # pruning grid cells per axis
NSAMPLE = 1024     # targets sampled for the per-query NN upper bound

_BF16 = ml_dtypes.bfloat16


# ---------------------------------------------------------------- host: pruning

def _kd_tiles(pts, n_tiles=NQT):
    idx = np.arange(len(pts))
    groups = [idx]
    while len(groups) < n_tiles:
        new = []
        for g in groups:
            p = pts[g]
            ax = int(np.argmax(p.max(0) - p.min(0)))
            order = np.argsort(p[:, ax], kind="stable")
            half = len(g) // 2
            new.append(g[order[:half]])
            new.append(g[order[half:]])
        groups = new
    return groups


def _candidates(Q, T):
    """Per tile: (query_idx [128], candidate_idx [w]). Guaranteed to contain
    each query's true nearest neighbor."""
    lo = T.min(0) - 1e-4
    hi = T.max(0) + 1e-4
    h = (hi - lo) / NG
    tcell = np.clip(((T - lo) / h), 0, NG - 1).astype(np.int64)
    tflat = (tcell[:, 0] * NG + tcell[:, 1]) * NG + tcell[:, 2]
    order = np.argsort(tflat, kind="stable")
    S = T[order[:: max(1, N // NSAMPLE)]].astype(np.float64)

    out = []
    for g in _kd_tiles(Q):
        tile = Q[g].astype(np.float64)
        d2 = ((tile[:, None, :] - S[None, :, :]) ** 2).sum(-1)
        ub = np.sqrt(d2.min(1)) * (1 + 1e-9) + 1e-9
        mark = np.zeros(NG * NG * NG, bool)
        m3 = mark.reshape(NG, NG, NG)
        lo_c = np.clip(((tile - ub[:, None] - lo) / h), 0, NG - 1).astype(np.int64)
        hi_c = np.clip(((tile + ub[:, None] - lo) / h), 0, NG - 1).astype(np.int64)
        for k in range(len(tile)):
            m3[lo_c[k, 0]:hi_c[k, 0] + 1,
               lo_c[k, 1]:hi_c[k, 1] + 1,
               lo_c[k, 2]:hi_c[k, 2] + 1] = True
        cand = np.nonzero(mark[tflat])[0]
        out.append((g, cand))
    return out


# ------------------------------------------------------------- host: operands

def _split3(x):
    x = x.astype(np.float32)
    h = x.astype(_BF16).astype(np.float32)
    m = (x - h).astype(_BF16).astype(np.float32)
    l = (x - h - m).astype(_BF16).astype(np.float32)
    return h, m, l


def _split5(x):
    out = []
    r = x.astype(np.float32).copy()
    for _ in range(5):
        h = r.astype(_BF16).astype(np.float32)
        out.append(h)
        r = r - h
    return out


def _build_rows(Q, T):
    """lhs [K, nq], rhs [K, nt] bf16-valued fp32 with
    sum_k lhs[k,q] * rhs[k,t] ~= |T_t|^2 - 2 Q_q . T_t  (fp32-level accuracy)."""
    qh, qm, ql = _split3(Q)
    th, tm, tl = _split3(T)
    t2 = (T.astype(np.float64) ** 2).sum(-1)
    t2l = _split5(t2.astype(np.float32))
    lhs, rhs = [], []
    for c in range(3):
        for (a, b) in ((qh, th), (qh, tm), (qm, th), (qh, tl), (ql, th),
                       (qm, tm), (qm, tl), (ql, tm), (ql, tl)):
            lhs.append(a[:, c])
            rhs.append(-2.0 * b[:, c])
    ones = np.ones(len(Q), np.float32)
    for t2i in t2l:
        lhs.append(ones)
        rhs.append(t2i)
    lhs = np.stack(lhs)
    rhs = np.stack(rhs)
    assert lhs.shape[0] == K and rhs.shape[0] == K
    return lhs, rhs


# ------------------------------------------------- custom DVE op (from baseline)

_MIN2_OP = None


def _get_min2_op():
    """Fused DVE op: out = min(in0, in1); accum_out = min(s0, min_k out[:, k]).
    One DVE pass examines TWO chunks (2 values per lane-cycle)."""
    global _MIN2_OP
    if _MIN2_OP is not None:
        return _MIN2_OP
    import re

    import numpy as np_

    from concourse import dve_ops
    from concourse.dve_spec import Spec, Src0, Src1, C0, minn

    name = "MIN2_REDUCE_CHAMFER"
    for op in dve_ops.OPS:
        if op.name == name:
            _MIN2_OP = op
            return op

    def _ref(in0, in1, s0, s1, imm2):
        out = np_.minimum(in0.astype(np_.float32), in1)
        acc = np_.minimum(out.min(axis=-1, keepdims=True), s0)
        return out, acc

    op = dve_ops.DveOp(
        name,
        Spec(body=minn(Src0, Src1), accum=minn, accum_init=C0, reference=_ref),
        subdim=False,
        uops_sha={},
    )
    dve_ops.OPS.append(op)
    dve_ops.CUSTOM_DVE_SPECS[name] = op.spec
    dve_ops._SUB_OPCODE_FOR_NAME[name] = dve_ops._CUSTOM_DVE_ROW_BASE + len(dve_ops.OPS) - 1
    assert max(dve_ops._SUB_OPCODE_FOR_NAME.values()) < 0x20
    for ver in ("v3", "v4"):
        try:
            op.compile(ver)
        except ValueError as e:
            m = re.search(rf"\({ver}: ([0-9a-f]+)", str(e))
            if m:
                op.uops_sha[ver] = m.group(1)
                op.compile(ver)
        except Exception:
            pass  # v4 lowering issues don't matter on TRN2
    _MIN2_OP = op
    return op


# ----------------------------------------------------- host: cross-core schedule

_SCHED = None   # tuple of per-entry chunk counts (SPMD-common)
_POST = None    # per-core postprocessing state


def _plan(src, tar):
    """Builds the SPMD-common entry schedule and per-core operand buffers."""
    cores = []
    for c in range(8):
        b, d = divmod(c, 2)
        Q, T = (src[b], tar[b]) if d == 0 else (tar[b], src[b])
        tiles = _candidates(Q, T)
        # split any tile with >MAXM chunks of candidates into multiple entries
        entries = []  # (qblock_index, cand_idx_list)
        for j, (g, cand) in enumerate(tiles):
            w = len(cand)
            pos = 0
            while True:
                take = min(w - pos, MAXM * CHUNK)
                entries.append((j, cand[pos:pos + take]))
                pos += take
                if pos >= w:
                    break
        entries.sort(key=lambda e: -len(e[1]))
        cores.append((Q, T, tiles, entries))

    n_entries = max(len(c[3]) for c in cores)
    mlist = []
    for r in range(n_entries):
        w = max(len(c[3][r][1]) if r < len(c[3]) else 1 for c in cores)
        mlist.append((w + CHUNK - 1) // CHUNK)

    in_maps, post = [], []
    for (Q, T, tiles, entries) in cores:
        lhs, rhs_full = _build_rows(Q, T)

        # lhs is ENTRY-indexed: entry r's 128 query columns live at
        # [r*128, (r+1)*128) — each core places its own tile there, so the
        # SPMD program can address lhs by entry id.
        qcols = []
        cols = []
        eq = []
        for r in range(n_entries):
            want = mlist[r] * CHUNK
            if r < len(entries):
                j, cand = entries[r]
            else:
                j, cand = 0, np.array([0])
            qcols.append(tiles[j][0])
            reps = -(-want // len(cand))
            cols.append(np.tile(cand, reps)[:want])
            eq.append(j)
        lhsT = np.ascontiguousarray(lhs[:, np.concatenate(qcols)])
        # duplicate into 4 PE row groups so LDWEIGHTS overlaps in-flight matmuls
        lhsT = np.tile(lhsT, (4, 1)).astype(_BF16)
        cols = np.concatenate(cols)
        rhs = np.ascontiguousarray(rhs_full[:, cols])
        rhs = np.tile(rhs, (4, 1)).astype(_BF16)

        q2 = (Q.astype(np.float64) ** 2).sum(-1)
        q2_tiled = np.stack([q2[g] for g, _ in tiles])  # [64, 128]
        init = np.full((QTILE, 2 * len(mlist)), 3.0e38, np.float32)
        in_maps.append({"lhs": lhsT, "rhs": rhs, "init": init})
        post.append({"entry_q": np.array(eq), "q2": q2_tiled})
    return tuple(mlist), in_maps, post


# ------------------------------------------------------------------ bass program

def _build_bass(mlist, repeat=1):
    _ensure_concourse()
    from contextlib import ExitStack

    import concourse.mybir as mybir
    import concourse.tile as tile
    from concourse import bacc

    E = len(mlist)
    W = sum(mlist) * CHUNK
    NQ = E * QTILE
    AMin = mybir.AluOpType.min
    min2 = _get_min2_op()

    nc = bacc.Bacc()
    lhs_d = nc.declare_dram_parameter("lhs", [4 * K, NQ], mybir.dt.bfloat16, isOutput=False)
    rhs_d = nc.declare_dram_parameter("rhs", [4 * K, W], mybir.dt.bfloat16, isOutput=False)
    init_d = nc.declare_dram_parameter("init", [QTILE, 2 * E], mybir.dt.float32, isOutput=False)
    out_d = nc.declare_dram_parameter("mins", [QTILE, 2 * E], mybir.dt.float32, isOutput=True)

    with ExitStack() as ctx:
        tc = ctx.enter_context(tile.TileContext(nc))
        singles = ctx.enter_context(tc.tile_pool(name="singles", bufs=1))
        psums = ctx.enter_context(tc.tile_pool(name="psums", bufs=2, space="PSUM"))
        stages = ctx.enter_context(tc.tile_pool(name="stages", bufs=3))
        scratch = ctx.enter_context(tc.tile_pool(name="scratch", bufs=2))

        lhs_s = singles.tile([4 * K, NQ], mybir.dt.bfloat16)
        rhs_s = singles.tile([4 * K, W], mybir.dt.bfloat16)
        allparts = singles.tile([QTILE, 2 * E], mybir.dt.float32)
        nc.sync.dma_start(out=allparts[:, :], in_=init_d[:, :])

        # input DMA, sliced so the first matmuls start before the tail arrives
        nc.sync.dma_start(out=lhs_s[:, 0:QTILE], in_=lhs_d[:, 0:QTILE])
        c0 = min(W, MAXM * CHUNK)
        nc.sync.dma_start(out=rhs_s[:, 0:c0], in_=rhs_d[:, 0:c0])
        nslice = 8
        step = -(-(W - c0) // nslice // CHUNK) * CHUNK
        pos = c0
        while pos < W:
            end = min(W, pos + step)
            nc.sync.dma_start(out=rhs_s[:, pos:end], in_=rhs_d[:, pos:end])
            pos = end
        nc.sync.dma_start(out=lhs_s[:, QTILE:NQ], in_=lhs_d[:, QTILE:NQ])

        def body():
            cnt = 0
            col = 0
            for r, m in enumerate(mlist):
                ps = psums.tile([QTILE, MAXM * CHUNK], mybir.dt.float32, name="ps")
                for kk in range(m):
                    g = (cnt % 4) * K
                    cnt += 1
                    nc.tensor.matmul(
                        ps[:, kk * CHUNK:(kk + 1) * CHUNK],
                        lhs_s[g:g + K, r * QTILE:(r + 1) * QTILE],
                        rhs_s[g:g + K, col:col + CHUNK],
                        start=True,
                        stop=True,
                        tile_position=(g, 0),
                    )
                    col += CHUNK
                if m == 1:
                    nc.vector.tensor_reduce(
                        allparts[:, 2 * r:2 * r + 1], ps[:, 0:CHUNK],
                        axis=mybir.AxisListType.X, op=AMin,
                    )
                else:
                    k = m // 2
                    nst = m - k
                    s = stages.tile([QTILE, 2 * CHUNK], mybir.dt.float32, name="s")
                    nc.scalar.copy(s[:, 0:nst * CHUNK], ps[:, k * CHUNK:m * CHUNK])
                    scr = scratch.tile([QTILE, 2 * CHUNK], mybir.dt.float32, name="scr")
                    nc.vector._custom_dve(
                        min2,
                        out=scr[:, 0:k * CHUNK],
                        in0=ps[:, 0:k * CHUNK],
                        in1=s[:, 0:k * CHUNK],
                        s0=3.0e38,
                        accum_out=allparts[:, 2 * r:2 * r + 1],
                    )
                    if nst > k:
                        nc.vector.tensor_reduce(
                            allparts[:, 2 * r + 1:2 * r + 2], s[:, k * CHUNK:nst * CHUNK],
                            axis=mybir.AxisListType.X, op=AMin,
                        )
            nc.sync.dma_start(out=out_d[:, :], in_=allparts)

        if repeat == 1:
            body()
        else:
            hint = (
                mybir.EngineType.PE,
                mybir.EngineType.DVE,
                mybir.EngineType.Activation,
                mybir.EngineType.SP,
            )
            with tc.For_i(0, repeat, 1, hint_engines=hint):
                body()
    nc.compile()
    return nc


_CACHED_NC = {}


def _get_nc(repeat=1, offload=True):
    key = (_SCHED, repeat)
    if key not in _CACHED_NC:
        _CACHED_NC[key] = _build_bass(_SCHED, repeat)
    return _CACHED_NC[key]


def run_cores(in_maps, trace=False):
    """Run the SPMD program on cores 0-7. Retries once after a pause (axon
    devices occasionally come up wedged after a crashed run)."""
    _ensure_concourse()
    import time as _time

    from concourse.bass_utils import run_bass_kernel_spmd

    nc = _get_nc()
    try:
        br = run_bass_kernel_spmd(nc, in_maps, list(range(8)), trace=trace)
    except Exception:
        _time.sleep(30)
        br = run_bass_kernel_spmd(nc, in_maps, list(range(8)), trace=trace)
    return br.results, br.exec_time_ns


def make_in_maps(src, tar):
    global _SCHED, _POST
    src = np.ascontiguousarray(np.asarray(src, dtype=np.float32))
    tar = np.ascontiguousarray(np.asarray(tar, dtype=np.float32))
    _SCHED, in_maps, _POST = _plan(src, tar)
    return in_maps


def postprocess(results):
    out = np.empty(B, np.float32)
    means = []
    for c in range(8):
        mins = results[c]["mins"].astype(np.float64)      # [128, 2E]
        emin = np.minimum(mins[:, 0::2], mins[:, 1::2])   # [128, E]
        st = _POST[c]
        nn = np.full((NQT, QTILE), np.inf)
        for r, j in enumerate(st["entry_q"]):
            nn[j] = np.minimum(nn[j], emin[:, r])
        d2 = np.maximum(nn + st["q2"], 0.0)
        means.append(np.sqrt(d2).mean())
    for b in range(B):
        out[b] = 0.5 * (means[2 * b] + means[2 * b + 1])
    return out


def kernel(src, tar):
    in_maps = make_in_maps(src, tar)
    results, _ = run_cores(in_maps, trace=False)
    return postprocess(results)


if __name__ == "__main__":
    rng = np.random.default_rng(0)
    src = rng.standard_normal((B, N, 3), dtype=np.float32)
    tar = rng.standard_normal((B, N, 3), dtype=np.float32)
    print(kernel(src, tar))


# revision 12
# speedup vs baseline: 4.4226x; 4.4226x over previous
"""Chamfer distance kernel for Trainium2 (8 NeuronCores) — pruned NN search.

Problem: src [4, 8192, 3], tar [4, 8192, 3] fp32 ->
    chamfer [4] = 0.5 * (mean_m ||src_m - NN(tar)|| + mean_n ||tar_n - NN(src)||)

Sharding: 8 cores = 4 batches x 2 directions; each core solves one 8192-query
nearest-neighbor problem against 8192 targets.

Algorithm (KD-tree-style pruning; the device computes every candidate
distance and all mins):
  host: kd-median-split queries into 64 tiles of 128; per query, an upper
  bound on its NN distance from a strided spatial sample of targets; the
  tile's candidate set = targets inside the union of per-query balls
  (rasterized on a 48^3 grid). Candidate sets provably contain the true NN.
  device: per (query-tile, candidate-chunk) entry, a K=32 bf16 limb matmul
  produces s = |t|^2 - 2 q.t in PSUM; VectorE tensor_tensor_reduce(min)
  folds PSUM chunks against ScalarE-staged chunks into per-entry mins.
  host: d2 = entry-min grouped per tile + |q|^2 (fp64), sqrt, mean.

Precision: 3-limb bf16 splits of coordinates with all 9 limb products kept
(27 rows) + 5-limb bf16 split of |t|^2 (5 rows) = K 32. |q|^2 is added on
the host in fp64, which removes the argmin-bias of the baseline
(~2.7e-4 rel err in simulation vs 1.6e-2 for the old kernel).
"""

import sys
import numpy as np
import ml_dtypes


def _ensure_concourse():
    try:
        import concourse.bass  # noqa: F401
    except ImportError:
        for p in ("/opt/trn_rl_repo", "/root/.axon_site/_ro/trn_rl_repo"):
            if p not in sys.path:
                sys.path.insert(0, p)
        import concourse.bass  # noqa: F401


B = 4
N = 8192
K = 32
QTILE = 128
NQT = N // QTILE   # 64 query tiles
CHUNK = 512        # candidate columns per matmul / PSUM bank
MAXM = 4           # chunks per entry (PSUM tile = 4 banks)
NG = 48            # pruning grid cells per axis
NSAMPLE = 1024     # targets sampled for the per-query NN upper bound

_BF16 = ml_dtypes.bfloat16


# ---------------------------------------------------------------- host: pruning

def _kd_tiles(pts, n_tiles=NQT):
    idx = np.arange(len(pts))
    groups = [idx]
    while len(groups) < n_tiles:
        new = []
        for g in groups:
            p = pts[g]
            ax = int(np.argmax(p.max(0) - p.min(0)))
            order = np.argsort(p[:, ax], kind="stable")
            half = len(g) // 2
            new.append(g[order[:half]])
            new.append(g[order[half:]])
        groups = new
    return groups


def _candidates(Q, T):
    """Per tile: (query_idx [128], candidate_idx [w]). Guaranteed to contain
    each query's true nearest neighbor."""
    lo = T.min(0) - 1e-4
    hi = T.max(0) + 1e-4
    h = (hi - lo) / NG
    tcell = np.clip(((T - lo) / h), 0, NG - 1).astype(np.int64)
    tflat = (tcell[:, 0] * NG + tcell[:, 1]) * NG + tcell[:, 2]
    order = np.argsort(tflat, kind="stable")
    S = T[order[:: max(1, N // NSAMPLE)]].astype(np.float64)

    out = []
    for g in _kd_tiles(Q):
        tile = Q[g].astype(np.float64)
        d2 = ((tile[:, None, :] - S[None, :, :]) ** 2).sum(-1)
        ub = np.sqrt(d2.min(1)) * (1 + 1e-9) + 1e-9
        mark = np.zeros(NG * NG * NG, bool)
        m3 = mark.reshape(NG, NG, NG)
        lo_c = np.clip(((tile - ub[:, None] - lo) / h), 0, NG - 1).astype(np.int64)
        hi_c = np.clip(((tile + ub[:, None] - lo) / h), 0, NG - 1).astype(np.int64)
        for k in range(len(tile)):
            m3[lo_c[k, 0]:hi_c[k, 0] + 1,
               lo_c[k, 1]:hi_c[k, 1] + 1,
               lo_c[k, 2]:hi_c[k, 2] + 1] = True
        cand = np.nonzero(mark[tflat])[0]
        out.append((g, cand))
    return out


# ------------------------------------------------------------- host: operands

def _split3(x):
    x = x.astype(np.float32)
    h = x.astype(_BF16).astype(np.float32)
    m = (x - h).astype(_BF16).astype(np.float32)
    l = (x - h - m).astype(_BF16).astype(np.float32)
    return h, m, l


def _split5(x):
    out = []
    r = x.astype(np.float32).copy()
    for _ in range(5):
        h = r.astype(_BF16).astype(np.float32)
        out.append(h)
        r = r - h
    return out


def _build_rows(Q, T):
    """lhs [K, nq], rhs [K, nt] bf16-valued fp32 with
    sum_k lhs[k,q] * rhs[k,t] ~= |T_t|^2 - 2 Q_q . T_t  (fp32-level accuracy)."""
    qh, qm, ql = _split3(Q)
    th, tm, tl = _split3(T)
    t2 = (T.astype(np.float64) ** 2).sum(-1)
    t2l = _split5(t2.astype(np.float32))
    lhs, rhs = [], []
    for c in range(3):
        for (a, b) in ((qh, th), (qh, tm), (qm, th), (qh, tl), (ql, th),
                       (qm, tm), (qm, tl), (ql, tm), (ql, tl)):
            lhs.append(a[:, c])
            rhs.append(-2.0 * b[:, c])
    ones = np.ones(len(Q), np.float32)
    for t2i in t2l:
        lhs.append(ones)
        rhs.append(t2i)
    lhs = np.stack(lhs)
    rhs = np.stack(rhs)
    assert lhs.shape[0] == K and rhs.shape[0] == K
    return lhs, rhs


# ------------------------------------------------- custom DVE op (from baseline)

_MIN2_OP = None


def _get_min2_op():
    """Fused DVE op: out = min(in0, in1); accum_out = min(s0, min_k out[:, k]).
    One DVE pass examines TWO chunks (2 values per lane-cycle)."""
    global _MIN2_OP
    if _MIN2_OP is not None:
        return _MIN2_OP
    import re

    import numpy as np_

    from concourse import dve_ops
    from concourse.dve_spec import Spec, Src0, Src1, C0, minn

    name = "MIN2_REDUCE_CHAMFER"
    for op in dve_ops.OPS:
        if op.name == name:
            _MIN2_OP = op
            return op

    def _ref(in0, in1, s0, s1, imm2):
        out = np_.minimum(in0.astype(np_.float32), in1)
        acc = np_.minimum(out.min(axis=-1, keepdims=True), s0)
        return out, acc

    op = dve_ops.DveOp(
        name,
        Spec(body=minn(Src0, Src1), accum=minn, accum_init=C0, reference=_ref),
        subdim=False,
        uops_sha={},
    )
    dve_ops.OPS.append(op)
    dve_ops.CUSTOM_DVE_SPECS[name] = op.spec
    dve_ops._SUB_OPCODE_FOR_NAME[name] = dve_ops._CUSTOM_DVE_ROW_BASE + len(dve_ops.OPS) - 1
    assert max(dve_ops._SUB_OPCODE_FOR_NAME.values()) < 0x20
    for ver in ("v3", "v4"):
        try:
            op.compile(ver)
        except ValueError as e:
            m = re.search(rf"\({ver}: ([0-9a-f]+)", str(e))
            if m:
                op.uops_sha[ver] = m.group(1)
                op.compile(ver)
        except Exception:
            pass  # v4 lowering issues don't matter on TRN2
    _MIN2_OP = op
    return op


# ----------------------------------------------------- host: cross-core schedule

_SCHED = None   # tuple of per-entry chunk counts (SPMD-common)
_POST = None    # per-core postprocessing state


def _plan(src, tar):
    """Builds the SPMD-common entry schedule and per-core operand buffers."""
    cores = []
    for c in range(8):
        b, d = divmod(c, 2)
        Q, T = (src[b], tar[b]) if d == 0 else (tar[b], src[b])
        tiles = _candidates(Q, T)
        # split any tile with >MAXM chunks of candidates into multiple entries
        entries = []  # (qblock_index, cand_idx_list)
        for j, (g, cand) in enumerate(tiles):
            w = len(cand)
            pos = 0
            while True:
                take = min(w - pos, MAXM * CHUNK)
                entries.append((j, cand[pos:pos + take]))
                pos += take
                if pos >= w:
                    break
        entries.sort(key=lambda e: -len(e[1]))
        cores.append((Q, T, tiles, entries))

    n_entries = max(len(c[3]) for c in cores)
    mlist = []
    for r in range(n_entries):
        w = max(len(c[3][r][1]) if r < len(c[3]) else 1 for c in cores)
        mlist.append((w + CHUNK - 1) // CHUNK)

    in_maps, post = [], []
    for (Q, T, tiles, entries) in cores:
        lhs, rhs_full = _build_rows(Q, T)

        # lhs is ENTRY-indexed: entry r's 128 query columns live at
        # [r*128, (r+1)*128) — each core places its own tile there, so the
        # SPMD program can address lhs by entry id.
        qcols = []
        cols = []
        eq = []
        for r in range(n_entries):
            want = mlist[r] * CHUNK
            if r < len(entries):
                j, cand = entries[r]
            else:
                j, cand = 0, np.array([0])
            qcols.append(tiles[j][0])
            reps = -(-want // len(cand))
            cols.append(np.tile(cand, reps)[:want])
            eq.append(j)
        lhsT = np.ascontiguousarray(lhs[:, np.concatenate(qcols)])
        # duplicate into 4 PE row groups so LDWEIGHTS overlaps in-flight matmuls
        lhsT = np.tile(lhsT, (4, 1)).astype(_BF16)
        cols = np.concatenate(cols)
        rhs = np.ascontiguousarray(rhs_full[:, cols])
        rhs = np.tile(rhs, (4, 1)).astype(_BF16)

        q2 = (Q.astype(np.float64) ** 2).sum(-1)
        q2_tiled = np.stack([q2[g] for g, _ in tiles])  # [64, 128]
        in_maps.append({"lhs": lhsT, "rhs": rhs})
        post.append({"entry_q": np.array(eq), "q2": q2_tiled})
    return tuple(mlist), in_maps, post


def _slots(mlist):
    """Slot layout mirror of _build_bass: entry -> (first_slot, n_slots)."""
    slot_of = []
    nslots = 0
    r = 0
    E = len(mlist)
    while r < E:
        if mlist[r] == 1:
            q = 1
            while q < MAXM and r + q < E and mlist[r + q] == 1:
                q += 1
            for i in range(q):
                slot_of.append((nslots + i, 1))
            nslots += q
            r += q
        else:
            m = mlist[r]
            ns = 2 if (m > 1 and m % 2 == 1) else 1
            slot_of.append((nslots, ns))
            nslots += ns
            r += 1
    return slot_of, nslots


# ------------------------------------------------------------------ bass program

def _build_bass(mlist, repeat=1):
    _ensure_concourse()
    from contextlib import ExitStack

    import concourse.mybir as mybir
    import concourse.tile as tile
    from concourse import bacc

    E = len(mlist)
    W = sum(mlist) * CHUNK
    NQ = E * QTILE
    AMin = mybir.AluOpType.min
    min2 = _get_min2_op()

    # slot map: every slot is written every iteration (no init needed), so the
    # output buffer can live in a rotating pool -> no cross-iteration WAR chain
    slot_of = []
    nslots = 0
    r = 0
    plan = []  # ("quad", [entries r..r+q)) for m==1 runs | ("one", r, m)
    while r < E:
        if mlist[r] == 1:
            q = 1
            while q < MAXM and r + q < E and mlist[r + q] == 1:
                q += 1
            plan.append(("quad", r, q))
            for i in range(q):
                slot_of.append((nslots + i, 1))
            nslots += q
            r += q
        else:
            m = mlist[r]
            ns = 2 if (m > 1 and m % 2 == 1) else 1
            plan.append(("one", r, m))
            slot_of.append((nslots, ns))
            nslots += ns
            r += 1

    nc = bacc.Bacc()
    lhs_d = nc.declare_dram_parameter("lhs", [4 * K, NQ], mybir.dt.bfloat16, isOutput=False)
    rhs_d = nc.declare_dram_parameter("rhs", [4 * K, W], mybir.dt.bfloat16, isOutput=False)
    out_d = nc.declare_dram_parameter("mins", [QTILE, nslots], mybir.dt.float32, isOutput=True)

    with ExitStack() as ctx:
        tc = ctx.enter_context(tile.TileContext(nc))
        singles = ctx.enter_context(tc.tile_pool(name="singles", bufs=1))
        psums = ctx.enter_context(tc.tile_pool(name="psums", bufs=2, space="PSUM"))
        stages = ctx.enter_context(tc.tile_pool(name="stages", bufs=3))
        scratch = ctx.enter_context(tc.tile_pool(name="scratch", bufs=2))
        partsp = ctx.enter_context(tc.tile_pool(name="partsp", bufs=2))

        lhs_s = singles.tile([4 * K, NQ], mybir.dt.bfloat16)
        rhs_s = singles.tile([4 * K, W], mybir.dt.bfloat16)

        # input DMA, sliced so the first matmuls start before the tail arrives
        nc.sync.dma_start(out=lhs_s[:, 0:QTILE], in_=lhs_d[:, 0:QTILE])
        c0 = min(W, MAXM * CHUNK)
        nc.sync.dma_start(out=rhs_s[:, 0:c0], in_=rhs_d[:, 0:c0])
        nslice = 8
        step = -(-(W - c0) // nslice // CHUNK) * CHUNK
        pos = c0
        while pos < W:
            end = min(W, pos + step)
            nc.sync.dma_start(out=rhs_s[:, pos:end], in_=rhs_d[:, pos:end])
            pos = end
        nc.sync.dma_start(out=lhs_s[:, QTILE:NQ], in_=lhs_d[:, QTILE:NQ])

        def body():
            allparts = partsp.tile([QTILE, nslots], mybir.dt.float32, name="allparts")
            cnt = 0
            col = 0
            for item in plan:
                kind, r = item[0], item[1]
                ps = psums.tile([QTILE, MAXM * CHUNK], mybir.dt.float32, name="ps")
                if kind == "quad":
                    q = item[2]
                    for i in range(q):
                        g = (cnt % 4) * K
                        cnt += 1
                        nc.tensor.matmul(
                            ps[:, i * CHUNK:(i + 1) * CHUNK],
                            lhs_s[g:g + K, (r + i) * QTILE:(r + i + 1) * QTILE],
                            rhs_s[g:g + K, col:col + CHUNK],
                            start=True, stop=True, tile_position=(g, 0),
                        )
                        col += CHUNK
                    s0 = slot_of[r][0]
                    nc.vector.tensor_reduce(
                        allparts[:, s0:s0 + q],
                        ps[:, 0:q * CHUNK].rearrange("p (i c) -> p i c", c=CHUNK),
                        axis=mybir.AxisListType.X, op=AMin,
                    )
                    continue
                m = item[2]
                for kk in range(m):
                    g = (cnt % 4) * K
                    cnt += 1
                    nc.tensor.matmul(
                        ps[:, kk * CHUNK:(kk + 1) * CHUNK],
                        lhs_s[g:g + K, r * QTILE:(r + 1) * QTILE],
                        rhs_s[g:g + K, col:col + CHUNK],
                        start=True, stop=True, tile_position=(g, 0),
                    )
                    col += CHUNK
                s0 = slot_of[r][0]
                k = m // 2
                nst = m - k
                s = stages.tile([QTILE, 2 * CHUNK], mybir.dt.float32, name="s")
                nc.scalar.copy(s[:, 0:nst * CHUNK], ps[:, k * CHUNK:m * CHUNK])
                scr = scratch.tile([QTILE, 2 * CHUNK], mybir.dt.float32, name="scr")
                nc.vector._custom_dve(
                    min2,
                    out=scr[:, 0:k * CHUNK],
                    in0=ps[:, 0:k * CHUNK],
                    in1=s[:, 0:k * CHUNK],
                    s0=3.0e38,
                    accum_out=allparts[:, s0:s0 + 1],
                )
                if nst > k:
                    nc.vector.tensor_reduce(
                        allparts[:, s0 + 1:s0 + 2], s[:, k * CHUNK:nst * CHUNK],
                        axis=mybir.AxisListType.X, op=AMin,
                    )
            nc.sync.dma_start(out=out_d[:, :], in_=allparts)

        if repeat == 1:
            body()
        else:
            hint = (
                mybir.EngineType.PE,
                mybir.EngineType.DVE,
                mybir.EngineType.Activation,
                mybir.EngineType.SP,
            )
            with tc.For_i(0, repeat, 1, hint_engines=hint):
                body()
    nc.compile()
    return nc


_CACHED_NC = {}


def _get_nc(repeat=1, offload=True):
    key = (_SCHED, repeat)
    if key not in _CACHED_NC:
        _CACHED_NC[key] = _build_bass(_SCHED, repeat)
    return _CACHED_NC[key]


def run_cores(in_maps, trace=False):
    """Run the SPMD program on cores 0-7. Retries once after a pause (axon
    devices occasionally come up wedged after a crashed run)."""
    _ensure_concourse()
    import time as _time

    from concourse.bass_utils import run_bass_kernel_spmd

    nc = _get_nc()
    try:
        br = run_bass_kernel_spmd(nc, in_maps, list(range(8)), trace=trace)
    except Exception:
        _time.sleep(30)
        br = run_bass_kernel_spmd(nc, in_maps, list(range(8)), trace=trace)
    return br.results, br.exec_time_ns


def make_in_maps(src, tar):
    global _SCHED, _POST
    src = np.ascontiguousarray(np.asarray(src, dtype=np.float32))
    tar = np.ascontiguousarray(np.asarray(tar, dtype=np.float32))
    _SCHED, in_maps, _POST = _plan(src, tar)
    return in_maps


def postprocess(results):
    out = np.empty(B, np.float32)
    means = []
    slot_of, _ = _slots(_SCHED)
    for c in range(8):
        mins = results[c]["mins"].astype(np.float64)      # [128, nslots]
        st = _POST[c]
        nn = np.full((NQT, QTILE), np.inf)
        for r, j in enumerate(st["entry_q"]):
            s0, ns = slot_of[r]
            emin = mins[:, s0:s0 + ns].min(1)
            nn[j] = np.minimum(nn[j], emin)
        d2 = np.maximum(nn + st["q2"], 0.0)
        means.append(np.sqrt(d2).mean())
    for b in range(B):
        out[b] = 0.5 * (means[2 * b] + means[2 * b + 1])
    return out


def kernel(src, tar):
    in_maps = make_in_maps(src, tar)
    results, _ = run_cores(in_maps, trace=False)
    return postprocess(results)


if __name__ == "__main__":
    rng = np.random.default_rng(0)
    src = rng.standard_normal((B, N, 3), dtype=np.float32)
    tar = rng.standard_normal((B, N, 3), dtype=np.float32)
    print(kernel(src, tar))
